# revision 21
# baseline (speedup 1.0000x reference)
"""Bass/Trainium2 kernel for nn_Conv2d_mvm (bit-sliced analog-crossbar conv2d).

The reference's bit-slice / bit-stream decomposition is mathematically lossless:
  - weight slices recombine exactly to wq = round(w * 256)            (int)
  - input bit-streams recombine exactly to patches = im2col(round(x*256))
so the whole model is exactly:
    out_int = conv2d(xq, wq, pad=1)               (int32, exact)
    out     = clip(out_int >> 4, -2^15, 2^15-1) / 4096 + bias

Ranges (verified): |xq| <= ~1224, |wq| <= ~89, |out_int| < 2^22.
Therefore fp16 operands with fp32 PSUM accumulation compute out_int exactly.

Sharding: data-parallel over batch, 1 image per NeuronCore (8 cores).

Per-core device pipeline (v3):
  1. Parallel input DMAs on both HWDGE queues (SP + ACT): padded x fp32
     [32,1156] in two column-halves, packed weights+bias [96,193] fp32.
  2. Quantize on device: xq = round_half_even(x*256) via the 1.5*2^23
     magic-number trick (exact RNE, matches np.round), fp16 out.
  3. Contract-dim packing, one DMA per kernel row r: an overlapping
     access pattern replicates xq three ways (shifts 34r+{0,1,2}) into a
     [96,1088] tile, so the 9-tap conv becomes 3 accumulating matmuls of
     contract 96 per spatial half.
  4. Postprocess per half: clip fused with the fp32->int32 convert
     (clip(v>>4) == clip(v, -2^19, 2^19-1) >> 4), arithmetic shift right
     4 (vector), then scale 1/4096 + per-channel bias on the scalar
     engine (int32 read, exact).
  5. Two output DMAs [64,512] (one per half, on separate queues).
"""

import numpy as np

import concourse.bass as bass
import concourse.mybir as mybir
import concourse.tile as tile
from concourse.bass_utils import run_bass_kernel_spmd

N_CORES = 8
MAGIC = 12582912.0  # 1.5 * 2**23: float add forces round-to-nearest-even int
CIN, COUT, H, W = 32, 64, 32, 32
PH, PW = H + 2, W + 2  # 34x34 padded
XCOLS = PH * PW        # 1156
NPIX = H * W           # 1024
RCOLS = 32 * PW        # 1088: replicated tile width
RLEN = 31 * PW + W     # 1086: columns actually needed per shifted copy

# tap t = di*3+dj reads padded pixel (oh+di, ow+dj) -> flat shift di*34+dj
SHIFTS = [di * PW + dj for di in range(3) for dj in range(3)]

# packed weight/bias buffer [128, 193] fp32:
#   cols   0- 63: lhsT_A (taps 0-3 stacked on partition blocks 32k)
#   cols  64-127: lhsT_B (taps 4-7)
#   cols 128-191: lhsT_C (tap 8, rows 0-31)
#   col  192    : bias (rows 0-63)
WB_COLS = 193

_CACHE = {}


def _split_multi_waits(nc):
    """TRN2 instructions encode at most ONE sync-wait command; Tile happily
    attaches one wait per producer proc (DMA lane / engine semaphore) to a
    consumer, which walrus rejects ("Too many sync wait commands").  Hoist
    the extra waits onto fresh single-wait NoOps inserted just before the
    instruction on the same engine (engine queues are in-order, so the
    semantics are identical)."""
    k = 0
    for f in nc.m.functions:
        for bb in f.blocks:
            insts = bb.instructions
            i = 0
            while i < len(insts):
                inst = insts[i]
                si = inst.sync_info
                if si is not None and len(si.on_wait) > 1:
                    waits = list(si.on_wait)
                    for w in waits[:-1]:
                        nop = mybir.InstNoOp(name=f"splitw_{k}", ins=[], outs=[])
                        k += 1
                        nop.engine = inst.engine
                        nop.sync_info = mybir.SyncInfo(on_wait=[w], on_update=[])
                        nc.register_instruction(nop)
                        insts.insert(i, nop)
                        i += 1
                    inst.sync_info = mybir.SyncInfo(
                        on_wait=[waits[-1]], on_update=list(si.on_update))
                i += 1
    return nc


def _build_module():
    nc = bass.Bass("TRN2", target_bir_lowering=False, debug=False)

    x_d = nc.dram_tensor("xpad", [CIN, XCOLS], mybir.dt.float32,
                         kind="ExternalInput")
    wb_d = nc.dram_tensor("wb", [128, WB_COLS], mybir.dt.float32,
                          kind="ExternalInput")
    y_d = nc.dram_tensor("y", [COUT, NPIX], mybir.dt.float32,
                         kind="ExternalOutput")

    AL = mybir.AluOpType
    F32, F16, I32 = mybir.dt.float32, mybir.dt.float16, mybir.dt.int32
    XH = XCOLS // 2  # 578

    with tile.TileContext(nc) as tc:
        from contextlib import ExitStack
        with ExitStack() as ctx:
            io = ctx.enter_context(tc.tile_pool(name="io", bufs=1))
            work = ctx.enter_context(tc.tile_pool(name="work", bufs=2))
            pp = ctx.enter_context(tc.tile_pool(name="psum", bufs=2, space="PSUM"))

            # --- input DMAs: split across the two HWDGE queues (SP, ACT) ---
            xt = io.tile([CIN, XCOLS], F32, tag="xt")
            nc.sync.dma_start(out=xt[:, 0:XH], in_=x_d[:, 0:XH])
            nc.scalar.dma_start(out=xt[:, XH:XCOLS], in_=x_d[:, XH:XCOLS])
            wb = io.tile([128, WB_COLS], F32, tag="wb")
            nc.sync.dma_start(out=wb[:], in_=wb_d[:])

            # weights fp32 -> fp16 (exact: small integers)
            wt = io.tile([128, 192], F16, tag="wt")
            nc.vector.tensor_copy(wt[:], wb[:, 0:192])
            b_ap = wb[0:COUT, 192:193]

            # --- quantize: xq = RNE(x*256) as fp16 (exact, |xq| < 2048) ---
            q1 = io.tile([CIN, XCOLS], F32, tag="q1")
            nc.vector.tensor_scalar(out=q1[:, 0:XH], in0=xt[:, 0:XH],
                                    scalar1=256.0, scalar2=MAGIC,
                                    op0=AL.mult, op1=AL.add)
            nc.vector.tensor_scalar(out=q1[:, XH:XCOLS], in0=xt[:, XH:XCOLS],
                                    scalar1=256.0, scalar2=MAGIC,
                                    op0=AL.mult, op1=AL.add)
            xq = io.tile([CIN, XCOLS], F16, tag="xq")
            nc.vector.tensor_scalar(out=xq[:], in0=q1[:], scalar1=-MAGIC,
                                    scalar2=None, op0=AL.add)

            # --- contract packing: plain per-block shifted copies of xq on
            # 128 partitions, spread across both HWDGE queues.  Group A =
            # taps 0-3, group B = taps 4-7, tap 8 reads xq directly.
            rA = io.tile([128, RCOLS], F16, tag="rA")
            rB = io.tile([128, RCOLS], F16, tag="rB")
            for blk in range(4):
                nc.sync.dma_start(
                    out=rA[32 * blk: 32 * blk + 32, 0:RLEN],
                    in_=xq[:, SHIFTS[blk]: SHIFTS[blk] + RLEN])
                nc.scalar.dma_start(
                    out=rB[32 * blk: 32 * blk + 32, 0:RLEN],
                    in_=xq[:, SHIFTS[4 + blk]: SHIFTS[4 + blk] + RLEN])
            rA3 = rA[:].rearrange("p (r c) -> p r c", c=PW)
            rB3 = rB[:].rearrange("p (r c) -> p r c", c=PW)
            xq3 = xq[:].rearrange("p (r c) -> p r c", c=PW)

            out_eng = [nc.scalar, nc.sync]
            for h in range(2):  # spatial halves: output rows [16h, 16h+16)
                ps = pp.tile([COUT, 512], F32, tag="ps")
                r0 = 16 * h
                nc.tensor.matmul(ps[:], wt[:, 0:64],
                                 rA3[:, r0:r0 + 16, 0:W],
                                 start=True, stop=False)
                nc.tensor.matmul(ps[:], wt[:, 64:128],
                                 rB3[:, r0:r0 + 16, 0:W],
                                 start=False, stop=False)
                nc.tensor.matmul(ps[:], wt[0:CIN, 128:192],
                                 xq3[:, 2 + r0: 2 + r0 + 16, 2:2 + W],
                                 start=False, stop=True)

                # clip fused with int convert: clip(v>>4, +-2^15) ==
                # clip(v, -2^19, 2^19-1) >> 4 ; psum values are exact ints
                c32 = work.tile([COUT, 512], I32, tag="c32")
                nc.vector.tensor_scalar(out=c32[:], in0=ps[:],
                                        scalar1=float((1 << 19) - 1),
                                        scalar2=float(-(1 << 19)),
                                        op0=AL.min, op1=AL.max)
                sf = work.tile([COUT, 512], I32, tag="sf")
                nc.vector.tensor_scalar(out=sf[:], in0=c32[:], scalar1=4,
                                        scalar2=None, op0=AL.arith_shift_right)
                # scalar engine: int32 -> fp32, q/4096 + bias (both exact)
                o = work.tile([COUT, 512], F32, tag="o")
                nc.scalar.activation(o[:], sf[:],
                                     mybir.ActivationFunctionType.Identity,
                                     bias=b_ap, scale=1.0 / 4096.0)
                out_eng[h].dma_start(out=y_d[:, 512 * h: 512 * (h + 1)],
                                     in_=o[:])

    return _split_multi_waits(nc)


def get_nc():
    if "nc" not in _CACHE:
        _CACHE["nc"] = _build_module()
    return _CACHE["nc"]


def prep_in_maps(x, weight, bias):
    x = np.asarray(x, dtype=np.float32)
    weight = np.asarray(weight, dtype=np.float32)
    bias = np.asarray(bias, dtype=np.float32)

    # weight quantization (host): wq = round_half_even(w*256); |wq| <= ~89
    wq = np.round(weight * np.float32(256.0)).astype(np.float32)
    # per tap (di,dj): lhsT[ci, co] = wq[co, ci, di, dj]
    taps = wq.transpose(1, 2, 3, 0).reshape(CIN, 9, COUT)  # [ci, t, co]

    wb = np.zeros((128, WB_COLS), dtype=np.float32)
    for blk in range(4):
        wb[32 * blk: 32 * blk + 32, 0:64] = taps[:, blk, :]
        wb[32 * blk: 32 * blk + 32, 64:128] = taps[:, 4 + blk, :]
    wb[0:CIN, 128:192] = taps[:, 8, :]
    wb[0:COUT, 192] = bias

    in_maps = []
    for c in range(N_CORES):
        xpad = np.pad(x[c], ((0, 0), (1, 1), (1, 1)))
        in_maps.append({
            "xpad": np.ascontiguousarray(xpad.reshape(CIN, XCOLS)),
            "wb": wb,
        })
    return in_maps


def run_spmd(in_maps, **kw):
    return run_bass_kernel_spmd(get_nc(), in_maps, list(range(N_CORES)), **kw)


def kernel(x, weight, bias):
    res = run_spmd(prep_in_maps(x, weight, bias))
    out = np.stack([r["y"].reshape(COUT, H, W) for r in res.results])
    return out.astype(np.float32)


# revision 22
# speedup vs baseline: 1.1683x; 1.1683x over previous
"""Bass/Trainium2 kernel for nn_Conv2d_mvm (bit-sliced analog-crossbar conv2d).

The reference's bit-slice / bit-stream decomposition is mathematically lossless:
  - weight slices recombine exactly to wq = round(w * 256)            (int)
  - input bit-streams recombine exactly to patches = im2col(round(x*256))
so the whole model is exactly:
    out_int = conv2d(xq, wq, pad=1)               (int32, exact)
    out     = clip(out_int >> 4, -2^15, 2^15-1) / 4096 + bias

Ranges (verified): |xq| <= ~1224, |wq| <= ~89, |out_int| < 2^22.
Therefore fp16 operands with fp32 PSUM accumulation compute out_int exactly.

Sharding: data-parallel over batch, 1 image per NeuronCore (8 cores).

Per-core device pipeline (v3):
  1. Parallel input DMAs on both HWDGE queues (SP + ACT): padded x fp32
     [32,1156] in two column-halves, packed weights+bias [96,193] fp32.
  2. Quantize on device: xq = round_half_even(x*256) via the 1.5*2^23
     magic-number trick (exact RNE, matches np.round), fp16 out.
  3. Contract-dim packing, one DMA per kernel row r: an overlapping
     access pattern replicates xq three ways (shifts 34r+{0,1,2}) into a
     [96,1088] tile, so the 9-tap conv becomes 3 accumulating matmuls of
     contract 96 per spatial half.
  4. Postprocess per half: clip fused with the fp32->int32 convert
     (clip(v>>4) == clip(v, -2^19, 2^19-1) >> 4), arithmetic shift right
     4 (vector), then scale 1/4096 + per-channel bias on the scalar
     engine (int32 read, exact).
  5. Two output DMAs [64,512] (one per half, on separate queues).
"""

import numpy as np

import concourse.bass as bass
import concourse.mybir as mybir
import concourse.tile as tile
from concourse.bass_utils import run_bass_kernel_spmd

N_CORES = 8
MAGIC = 12582912.0  # 1.5 * 2**23: float add forces round-to-nearest-even int
CIN, COUT, H, W = 32, 64, 32, 32
PH, PW = H + 2, W + 2  # 34x34 padded
XCOLS = PH * PW        # 1156
NPIX = H * W           # 1024
RCOLS = 32 * PW        # 1088: replicated tile width
RLEN = 31 * PW + W     # 1086: columns actually needed per shifted copy

# tap t = di*3+dj reads padded pixel (oh+di, ow+dj) -> flat shift di*34+dj
SHIFTS = [di * PW + dj for di in range(3) for dj in range(3)]

# packed weight/bias buffer [128, 193] fp32:
#   cols   0- 63: lhsT_A (taps 0-3 stacked on partition blocks 32k)
#   cols  64-127: lhsT_B (taps 4-7)
#   cols 128-191: lhsT_C (tap 8, rows 0-31)
#   col  192    : bias (rows 0-63)
WB_COLS = 193

_CACHE = {}


def _split_multi_waits(nc):
    """TRN2 instructions encode at most ONE sync-wait command; Tile happily
    attaches one wait per producer proc (DMA lane / engine semaphore) to a
    consumer, which walrus rejects ("Too many sync wait commands").  Hoist
    the extra waits onto fresh single-wait NoOps inserted just before the
    instruction on the same engine (engine queues are in-order, so the
    semantics are identical)."""
    k = 0
    for f in nc.m.functions:
        for bb in f.blocks:
            insts = bb.instructions
            i = 0
            while i < len(insts):
                inst = insts[i]
                si = inst.sync_info
                if si is not None and len(si.on_wait) > 1:
                    waits = list(si.on_wait)
                    for w in waits[:-1]:
                        nop = mybir.InstNoOp(name=f"splitw_{k}", ins=[], outs=[])
                        k += 1
                        nop.engine = inst.engine
                        nop.sync_info = mybir.SyncInfo(on_wait=[w], on_update=[])
                        nc.register_instruction(nop)
                        insts.insert(i, nop)
                        i += 1
                    inst.sync_info = mybir.SyncInfo(
                        on_wait=[waits[-1]], on_update=list(si.on_update))
                i += 1
    return nc


def _build_module():
    nc = bass.Bass("TRN2", target_bir_lowering=False, debug=False)

    x_d = nc.dram_tensor("xpad", [CIN, XCOLS], mybir.dt.float32,
                         kind="ExternalInput")
    wb_d = nc.dram_tensor("wb", [128, WB_COLS], mybir.dt.float32,
                          kind="ExternalInput")
    y_d = nc.dram_tensor("y", [COUT, NPIX], mybir.dt.float32,
                         kind="ExternalOutput")

    AL = mybir.AluOpType
    F32, F16, I32 = mybir.dt.float32, mybir.dt.float16, mybir.dt.int32
    XH = XCOLS // 2  # 578

    with tile.TileContext(nc) as tc:
        from contextlib import ExitStack
        with ExitStack() as ctx:
            io = ctx.enter_context(tc.tile_pool(name="io", bufs=1))
            work = ctx.enter_context(tc.tile_pool(name="work", bufs=2))
            pp = ctx.enter_context(tc.tile_pool(name="psum", bufs=2, space="PSUM"))

            # --- input DMAs ---
            xt = io.tile([CIN, XCOLS], F32, tag="xt")
            nc.sync.dma_start(out=xt[:], in_=x_d[:])
            wb = io.tile([128, WB_COLS], F32, tag="wb")
            nc.scalar.dma_start(out=wb[:], in_=wb_d[:])

            # weights fp32 -> fp16 (exact: small integers)
            wt = io.tile([128, 192], F16, tag="wt")
            nc.vector.tensor_copy(wt[:], wb[:, 0:192])
            b_ap = wb[0:COUT, 192:193]

            # --- quantize: xq = RNE(x*256) as fp16 (exact, |xq| < 2048) ---
            q1 = io.tile([CIN, XCOLS], F32, tag="q1")
            nc.vector.tensor_scalar(out=q1[:], in0=xt[:], scalar1=256.0,
                                    scalar2=MAGIC, op0=AL.mult, op1=AL.add)
            xq = io.tile([CIN, XCOLS], F16, tag="xq")
            nc.vector.tensor_scalar(out=xq[:], in0=q1[:], scalar1=-MAGIC,
                                    scalar2=None, op0=AL.add)

            # --- contract packing: per-block shifted copies of xq on 128
            # partitions.  Group A (taps 0-3) copies on the DVE (fp16 4x
            # copy mode) so the first matmuls start early; group B
            # (taps 4-7) on the DMA queues, overlapped with A's matmuls;
            # tap 8 reads xq directly.
            rA = io.tile([128, RCOLS], F16, tag="rA")
            rB = io.tile([128, RCOLS], F16, tag="rB")
            for blk in range(4):
                nc.vector.tensor_copy(
                    rA[32 * blk: 32 * blk + 32, 0:RLEN],
                    xq[:, SHIFTS[blk]: SHIFTS[blk] + RLEN])
                (nc.sync if blk % 2 == 0 else nc.scalar).dma_start(
                    out=rB[32 * blk: 32 * blk + 32, 0:RLEN],
                    in_=xq[:, SHIFTS[4 + blk]: SHIFTS[4 + blk] + RLEN])
            rA3 = rA[:].rearrange("p (r c) -> p r c", c=PW)
            rB3 = rB[:].rearrange("p (r c) -> p r c", c=PW)
            xq3 = xq[:].rearrange("p (r c) -> p r c", c=PW)

            # interleave the two accumulation groups (one PSUM bank per
            # spatial half) so group B / tap C inputs get extra slack
            ps0 = pp.tile([COUT, 512], F32, tag="ps", name="ps0")
            ps1 = pp.tile([COUT, 512], F32, tag="ps", name="ps1")
            pss = [ps0, ps1]
            for h in range(2):
                nc.tensor.matmul(pss[h][:], wt[:, 0:64],
                                 rA3[:, 16 * h: 16 * h + 16, 0:W],
                                 start=True, stop=False)
            for h in range(2):
                nc.tensor.matmul(pss[h][:], wt[:, 64:128],
                                 rB3[:, 16 * h: 16 * h + 16, 0:W],
                                 start=False, stop=False)
            out_eng = [nc.scalar, nc.sync]
            for h in range(2):  # spatial halves: output rows [16h, 16h+16)
                ps = pss[h]
                nc.tensor.matmul(ps[:], wt[0:CIN, 128:192],
                                 xq3[:, 2 + 16 * h: 2 + 16 * h + 16, 2:2 + W],
                                 start=False, stop=True)

                # clip fused with int convert: clip(v>>4, +-2^15) ==
                # clip(v, -2^19, 2^19-1) >> 4 ; psum values are exact ints
                c32 = work.tile([COUT, 512], I32, tag="c32")
                nc.vector.tensor_scalar(out=c32[:], in0=ps[:],
                                        scalar1=float((1 << 19) - 1),
                                        scalar2=float(-(1 << 19)),
                                        op0=AL.min, op1=AL.max)
                sf = work.tile([COUT, 512], I32, tag="sf")
                nc.vector.tensor_scalar(out=sf[:], in0=c32[:], scalar1=4,
                                        scalar2=None, op0=AL.arith_shift_right)
                # scalar engine: int32 -> fp32, q/4096 + bias (both exact)
                o = work.tile([COUT, 512], F32, tag="o")
                nc.scalar.activation(o[:], sf[:],
                                     mybir.ActivationFunctionType.Identity,
                                     bias=b_ap, scale=1.0 / 4096.0)
                out_eng[h].dma_start(out=y_d[:, 512 * h: 512 * (h + 1)],
                                     in_=o[:])

    return _split_multi_waits(nc)


def get_nc():
    if "nc" not in _CACHE:
        _CACHE["nc"] = _build_module()
    return _CACHE["nc"]


def prep_in_maps(x, weight, bias):
    x = np.asarray(x, dtype=np.float32)
    weight = np.asarray(weight, dtype=np.float32)
    bias = np.asarray(bias, dtype=np.float32)

    # weight quantization (host): wq = round_half_even(w*256); |wq| <= ~89
    wq = np.round(weight * np.float32(256.0)).astype(np.float32)
    # per tap (di,dj): lhsT[ci, co] = wq[co, ci, di, dj]
    taps = wq.transpose(1, 2, 3, 0).reshape(CIN, 9, COUT)  # [ci, t, co]

    wb = np.zeros((128, WB_COLS), dtype=np.float32)
    for blk in range(4):
        wb[32 * blk: 32 * blk + 32, 0:64] = taps[:, blk, :]
        wb[32 * blk: 32 * blk + 32, 64:128] = taps[:, 4 + blk, :]
    wb[0:CIN, 128:192] = taps[:, 8, :]
    wb[0:COUT, 192] = bias

    in_maps = []
    for c in range(N_CORES):
        xpad = np.pad(x[c], ((0, 0), (1, 1), (1, 1)))
        in_maps.append({
            "xpad": np.ascontiguousarray(xpad.reshape(CIN, XCOLS)),
            "wb": wb,
        })
    return in_maps


def run_spmd(in_maps, **kw):
    return run_bass_kernel_spmd(get_nc(), in_maps, list(range(N_CORES)), **kw)


def kernel(x, weight, bias):
    res = run_spmd(prep_in_maps(x, weight, bias))
    out = np.stack([r["y"].reshape(COUT, H, W) for r in res.results])
    return out.astype(np.float32)
